# revision 5
# baseline (speedup 1.0000x reference)
"""CMaxPool4d (complex modulus max-pool, K=2 stride 2 over 4 spatial dims) on 8 Trainium2 cores.

Input  : [8, 2, 32, 16, 16, 16, 16] f32  (dim1 = real/imag)
Output : [8, 2, 32, 8, 8, 8, 8] f32      (value of r/i at the max-|z| position of each 2^4 window)

Strategy: data-parallel over batch (core b <- batch b). Per core, a 4-level
pairwise tournament over the 16 window candidates, LSB-first (d4, d3, d2, d1)
so ties resolve to the first (lowest) window index, matching jnp.argmax.
Each merge: mask = (m_hi > m_lo) on DVE; payload (r,i) moved in place with
copy_predicated; m updated with max. m = r^2 + i^2 (ACT Square + DVE add);
sqrt is monotone so it is not needed for the comparison.

Host pre-permutes the input so every per-chunk DMA is one contiguous 4 MiB
transfer; host post-permutes the 8 per-core outputs back to the full shape.
"""

import os
import sys

import numpy as np

for p in ("/opt/trn_rl_repo", "/opt/pypackages", "/root/.axon_site", "/root/.axon_site/_ro/trn_rl_repo", "/root/.axon_site/_ro/pypackages"):
    if os.path.isdir(p) and p not in sys.path:
        sys.path.append(p)

from concourse import bacc, mybir  # noqa: E402
from concourse.tile import TileContext  # noqa: E402
from concourse.bass_utils import run_bass_kernel_spmd  # noqa: E402

N_CORES = 8
RI = 2          # real/imag
C = 32          # channels per core
D = 16          # each spatial dim
O = D // 2      # pooled spatial dim
NCH = 8         # channels per chunk
NCHUNK = int(os.environ.get("K_NCHUNK", C // NCH))  # 4 normally
SLAB = NCH * O * O * D * D // 128  # free elems per slab per partition = 1024
XF = 8 * SLAB        # X tile free dim = 8192
MF = 4 * SLAB // 2   # M tile free dim half?  (4 blocks of 1024) = 4096 incorrect name; see below

F32 = mybir.dt.float32
U8 = mybir.dt.uint8

# engine routing (tuned from profiling): which engine runs the m-adds and
# each level's m-max. DVE is the bottleneck; GPSIMD runs tensor_tensor at
# ~half DVE rate but fully in parallel (no port contention with 1x DVE ops).
ADD_ENGINE = os.environ.get("K_ADD_ENGINE", "gpsimd")
MAX4_ENGINE = os.environ.get("K_MAX4_ENGINE", "gpsimd")
MAX3_ENGINE = os.environ.get("K_MAX3_ENGINE", "vector")
MAX2_ENGINE = os.environ.get("K_MAX2_ENGINE", "vector")

_COMPILED = None  # cache (nc)


def _build():
    nc = bacc.Bacc("TRN2", num_devices=N_CORES)
    x_dram = nc.declare_dram_parameter("x", [NCHUNK, 128, XF], F32, isOutput=False)
    y_dram = nc.declare_dram_parameter("y", [NCHUNK, 128, 512], F32, isOutput=True)

    with TileContext(nc) as tc:
        with tc.tile_pool(name="sbuf", bufs=2) as pool:
            for k in range(NCHUNK):
                X = pool.tile([128, XF], F32, tag="X")
                nc.sync.dma_start(out=X[:, :], in_=x_dram[k])

                # ---- m = r^2 + i^2 (per half a=0,1) ----
                M = pool.tile([128, 4096], F32, tag="M")  # 4 blocks t=(a,b) of 1024
                for h in range(2):
                    SQ = pool.tile([128, 4096], F32, tag="SQ")
                    nc.scalar.activation(
                        SQ[:, :], X[:, h * 4096:(h + 1) * 4096],
                        mybir.ActivationFunctionType.Square,
                    )
                    sqv = SQ.rearrange("p (b ri f) -> p b ri f", b=2, ri=2)
                    mv = M.rearrange("p (t f) -> p t f", t=4)
                    getattr(nc, ADD_ENGINE).tensor_tensor(
                        mv[:, 2 * h:2 * h + 2], sqv[:, :, 0], sqv[:, :, 1],
                        mybir.AluOpType.add,
                    )

                # ---- level D4 (innermost): pairs adjacent d4 ----
                mz = M.rearrange("p (z d) -> p z d", d=2)
                mask4 = pool.tile([128, 2048], U8, tag="mask4")
                nc.vector.tensor_tensor(mask4[:, :], mz[:, :, 1], mz[:, :, 0], mybir.AluOpType.is_gt)
                xz = X.rearrange("p (t ri z d) -> p t ri z d", t=4, ri=2, d=2)
                m4t = mask4.rearrange("p (t z) -> p t z", t=4)
                for t in range(4):
                    mk = m4t[:, t].unsqueeze(1).broadcast_to((128, 2, 512))
                    nc.vector.copy_predicated(xz[:, t, :, :, 0], mk, xz[:, t, :, :, 1])
                getattr(nc, MAX4_ENGINE).tensor_tensor(mz[:, :, 0], mz[:, :, 0], mz[:, :, 1], mybir.AluOpType.max)

                # ---- level D3: pairs adjacent d3 (s bit), valid at d=0 ----
                # M block-local: v = u*32 + s*16 + o4*2 + d   (u = (q,o3), 128 over 4 blocks)
                mu = M.rearrange("p (u s o4 d) -> p u s o4 d", s=2, o4=8, d=2)
                mask3 = pool.tile([128, 1024], U8, tag="mask3")
                nc.vector.tensor_tensor(
                    mask3[:, :], mu[:, :, 1, :, 0], mu[:, :, 0, :, 0], mybir.AluOpType.is_gt
                )
                xu = X.rearrange("p (t ri u s o4 d) -> p t ri u s o4 d", t=4, ri=2, u=32, s=2, o4=8, d=2)
                m3t = mask3.rearrange("p (t u o4) -> p t u o4", t=4, u=32)
                for t in range(4):
                    mk = m3t[:, t].unsqueeze(1).broadcast_to((128, 2, 32, 8))
                    nc.vector.copy_predicated(
                        xu[:, t, :, :, 0, :, 0], mk, xu[:, t, :, :, 1, :, 0]
                    )
                getattr(nc, MAX3_ENGINE).tensor_tensor(
                    mu[:, :, 0, :, 0], mu[:, :, 0, :, 0], mu[:, :, 1, :, 0], mybir.AluOpType.max
                )

                # ---- level D2: pairs b (t low bit), valid at s=0, d=0 ----
                mab = M.rearrange("p (a b u s o4 d) -> p a b u s o4 d", a=2, b=2, u=32, s=2, o4=8, d=2)
                mask2 = pool.tile([128, 512], U8, tag="mask2")
                nc.vector.tensor_tensor(
                    mask2[:, :], mab[:, :, 1, :, 0, :, 0], mab[:, :, 0, :, 0, :, 0],
                    mybir.AluOpType.is_gt,
                )
                xab = X.rearrange(
                    "p (a b ri u s o4 d) -> p a b ri u s o4 d", a=2, b=2, ri=2, u=32, s=2, o4=8, d=2
                )
                m2t = mask2.rearrange("p (a u o4) -> p a u o4", a=2, u=32)
                for a in range(2):
                    mk = m2t[:, a].unsqueeze(1).broadcast_to((128, 2, 32, 8))
                    nc.vector.copy_predicated(
                        xab[:, a, 0, :, :, 0, :, 0], mk, xab[:, a, 1, :, :, 0, :, 0]
                    )
                getattr(nc, MAX2_ENGINE).tensor_tensor(
                    mab[:, :, 0, :, 0, :, 0], mab[:, :, 0, :, 0, :, 0], mab[:, :, 1, :, 0, :, 0],
                    mybir.AluOpType.max,
                )

                # ---- level D1: pairs a, valid at b=0, s=0, d=0; no m update ----
                mask1 = pool.tile([128, 256], U8, tag="mask1")
                nc.vector.tensor_tensor(
                    mask1[:, :], mab[:, 1, 0, :, 0, :, 0], mab[:, 0, 0, :, 0, :, 0],
                    mybir.AluOpType.is_gt,
                )
                m1t = mask1.rearrange("p (u o4) -> p u o4", u=32)
                mk = m1t.unsqueeze(1).broadcast_to((128, 2, 32, 8))
                nc.vector.copy_predicated(
                    xab[:, 0, 0, :, :, 0, :, 0], mk, xab[:, 1, 0, :, :, 0, :, 0]
                )

                # ---- compact winners (slabs j=0 r, j=1 i) to dense OUT and store ----
                OUT = pool.tile([128, 512], F32, tag="OUT")
                outv = OUT.rearrange("p (ri u o4) -> p ri u o4", ri=2, u=32)
                nc.scalar.activation(
                    outv, xab[:, 0, 0, :, :, 0, :, 0],
                    mybir.ActivationFunctionType.Copy,
                )
                nc.sync.dma_start(out=y_dram[k], in_=OUT[:, :])

    nc.compile()
    return nc


def _get_nc():
    global _COMPILED
    if _COMPILED is None:
        _COMPILED = _build()
    return _COMPILED


def _prep_core(xb: np.ndarray) -> np.ndarray:
    """xb: [2, 32, 16,16,16,16] -> [NCHUNK, 128, 8192] slab-packed."""
    # [ri, chunk, c8, o1, a, o2, bq, d3, d4]
    t = xb.reshape(RI, C // NCH, NCH, O, 2, O, 2, D, D)
    # -> [chunk, a, bq, ri, c8, o1, o2, d3, d4]
    t = t.transpose(1, 4, 6, 0, 2, 3, 5, 7, 8)
    # slab content [c8, o1, o2, d3, d4] = 131072 = [128, 1024]
    t = t.reshape(C // NCH, 2, 2, RI, 128, SLAB)
    # -> [chunk, p, a, bq, ri, f]
    t = t.transpose(0, 4, 1, 2, 3, 5)
    return np.ascontiguousarray(t).reshape(C // NCH, 128, XF)


def _post_core(y: np.ndarray) -> np.ndarray:
    """y: [NCHUNK, 128, 512] -> [2, 32, 8, 8, 8, 8]."""
    yk = y.reshape(C // NCH, 128, RI, 256)
    out = yk.transpose(2, 0, 1, 3).reshape(RI, C, O, O, O, O)
    return out


def _run(inputs_x: np.ndarray, trace: bool = False):
    nc = _get_nc()
    in_maps = [{"x": _prep_core(inputs_x[b])} for b in range(N_CORES)]
    last_err = None
    for _attempt in range(3):
        try:
            res = run_bass_kernel_spmd(nc, in_maps, list(range(N_CORES)), trace=trace)
            break
        except Exception as e:  # wedged-device retries
            last_err = e
            if "UNRECOVERABLE" not in str(e) and "UNAVAILABLE" not in str(e):
                raise
    else:
        raise last_err
    outs = np.empty((N_CORES, RI, C, O, O, O, O), dtype=np.float32)
    for b in range(N_CORES):
        outs[b] = _post_core(res.results[b]["y"])
    return outs, res


def kernel(input: np.ndarray) -> np.ndarray:
    input = np.asarray(input, dtype=np.float32)
    outs, _ = _run(input)
    return outs


# revision 7
# speedup vs baseline: 4.2223x; 4.2223x over previous
"""CMaxPool4d (complex modulus max-pool, K=2 stride 2 over 4 spatial dims) on 8 Trainium2 cores.

Input  : [8, 2, 32, 16, 16, 16, 16] f32  (dim1 = real/imag)
Output : [8, 2, 32, 8, 8, 8, 8] f32      (value of r/i at the max-|z| position of each 2^4 window)

Strategy: data-parallel over batch (core b <- batch b). Per core, a 4-level
pairwise tournament over the 16 window candidates, LSB-first (d4, d3, d2, d1)
so ties resolve to the first (lowest) window index, matching jnp.argmax.
Each merge: mask = (m_hi > m_lo) on DVE; payload (r,i) moved in place with
copy_predicated; m updated with max. m = r^2 + i^2 (ACT Square + add);
sqrt is monotone so it is not needed for the comparison.

The host pre-permute splits the d4/d3 parities into separate contiguous
blocks (slab-local f = d*512 + s*256 + q*64 + o3*8 + o4), so every engine op
reads/writes contiguous runs and GPSIMD (flat-1D-AP-only) can take the adds
and one max level. Each chunk is one contiguous 4 MiB DMA; winners land at
the front of slabs j=0/1 and are DMAed out directly (no compaction pass).
"""

import os
import sys

import numpy as np

for p in ("/opt/trn_rl_repo", "/opt/pypackages", "/root/.axon_site", "/root/.axon_site/_ro/trn_rl_repo", "/root/.axon_site/_ro/pypackages"):
    if os.path.isdir(p) and p not in sys.path:
        sys.path.append(p)

from concourse import bacc, mybir  # noqa: E402
from concourse.tile import TileContext  # noqa: E402
from concourse.bass_utils import run_bass_kernel_spmd  # noqa: E402

N_CORES = 8
RI = 2
C = 32
D = 16
O = D // 2
NCH = 8                    # channels per chunk
NCHUNK = int(os.environ.get("K_NCHUNK", C // NCH))
SLAB = 1024                # free elems per slab per partition
XF = 8 * SLAB              # 8192

F32 = mybir.dt.float32
U8 = mybir.dt.uint8

ADD_ENGINE = os.environ.get("K_ADD_ENGINE", "gpsimd")
MAX4_ENGINE = os.environ.get("K_MAX4_ENGINE", "vector")
MAX3_ENGINE = os.environ.get("K_MAX3_ENGINE", "vector")
MAX2_ENGINE = os.environ.get("K_MAX2_ENGINE", "vector")

_COMPILED = None


def _build():
    nc = bacc.Bacc("TRN2", num_devices=N_CORES)
    x_dram = nc.declare_dram_parameter("x", [NCHUNK, 128, XF], F32, isOutput=False)
    y_dram = nc.declare_dram_parameter("y", [NCHUNK, 128, 512], F32, isOutput=True)

    with TileContext(nc) as tc:
        with tc.tile_pool(name="sbuf", bufs=2) as pool:
            for k in range(NCHUNK):
                X = pool.tile([128, XF], F32, tag="X")
                nc.sync.dma_start(out=X[:, :], in_=x_dram[k])

                xtr = X.rearrange("p (t ri f) -> p t ri f", t=4, ri=2)

                # ---- m = r^2 + i^2; SQ written ri-major so the adds are flat ----
                M = pool.tile([128, 4096], F32, tag="M")  # 4 t-blocks of 1024
                for h in range(2):
                    SQ = pool.tile([128, 4096], F32, tag="SQ")
                    nc.scalar.activation(
                        SQ.rearrange("p (ri b f) -> p b ri f", ri=2, b=2),
                        X[:, h * 4096:(h + 1) * 4096],
                        mybir.ActivationFunctionType.Square,
                    )
                    getattr(nc, ADD_ENGINE).tensor_tensor(
                        M[:, h * 2048:(h + 1) * 2048], SQ[:, 0:2048], SQ[:, 2048:4096],
                        mybir.AluOpType.add,
                    )

                mt = M.rearrange("p (t f) -> p t f", t=4)

                def max_level(engine, n_t, t_step, half):
                    if engine == "gpsimd":
                        for t in range(0, n_t * t_step, t_step):
                            base = t * SLAB
                            nc.gpsimd.tensor_tensor(
                                M[:, base:base + half], M[:, base:base + half],
                                M[:, base + half:base + 2 * half], mybir.AluOpType.max,
                            )
                    else:
                        v = mt[:, ::t_step] if t_step > 1 else mt
                        nc.vector.tensor_tensor(
                            v[:, :n_t, 0:half], v[:, :n_t, 0:half],
                            v[:, :n_t, half:2 * half], mybir.AluOpType.max,
                        )

                # ---- D4: d pairs; even block [0:512), odd [512:1024) per t ----
                mask4 = pool.tile([128, 2048], U8, tag="mask4")
                m4 = mask4.rearrange("p (t f) -> p t f", t=4)
                nc.vector.tensor_tensor(
                    m4, mt[:, :, 512:1024], mt[:, :, 0:512], mybir.AluOpType.is_gt
                )
                for t in range(4):
                    mk = m4[:, t].unsqueeze(1).broadcast_to((128, 2, 512))
                    nc.vector.copy_predicated(
                        xtr[:, t, :, 0:512], mk, xtr[:, t, :, 512:1024]
                    )
                max_level(MAX4_ENGINE, 4, 1, 512)

                # ---- D3: s pairs; [0:256) vs [256:512) per t ----
                mask3 = pool.tile([128, 1024], U8, tag="mask3")
                m3 = mask3.rearrange("p (t f) -> p t f", t=4)
                nc.vector.tensor_tensor(
                    m3, mt[:, :, 256:512], mt[:, :, 0:256], mybir.AluOpType.is_gt
                )
                for t in range(4):
                    mk = m3[:, t].unsqueeze(1).broadcast_to((128, 2, 256))
                    nc.vector.copy_predicated(
                        xtr[:, t, :, 0:256], mk, xtr[:, t, :, 256:512]
                    )
                max_level(MAX3_ENGINE, 4, 1, 256)

                # ---- D2: b pairs (t odd vs t even); [0:256) ----
                ma = M.rearrange("p (a b f) -> p a b f", a=2, b=2)
                mask2 = pool.tile([128, 512], U8, tag="mask2")
                m2 = mask2.rearrange("p (a f) -> p a f", a=2)
                nc.vector.tensor_tensor(
                    m2, ma[:, :, 1, 0:256], ma[:, :, 0, 0:256], mybir.AluOpType.is_gt
                )
                for a in range(2):
                    mk = m2[:, a].unsqueeze(1).broadcast_to((128, 2, 256))
                    nc.vector.copy_predicated(
                        xtr[:, 2 * a, :, 0:256], mk, xtr[:, 2 * a + 1, :, 0:256]
                    )
                if MAX2_ENGINE == "gpsimd":
                    for a in range(2):
                        base = 2 * a * SLAB
                        nc.gpsimd.tensor_tensor(
                            M[:, base:base + 256], M[:, base:base + 256],
                            M[:, base + SLAB:base + SLAB + 256], mybir.AluOpType.max,
                        )
                else:
                    nc.vector.tensor_tensor(
                        ma[:, :, 0, 0:256], ma[:, :, 0, 0:256], ma[:, :, 1, 0:256],
                        mybir.AluOpType.max,
                    )

                # ---- D1: a pairs (t=2 vs t=0); no m update ----
                mask1 = pool.tile([128, 256], U8, tag="mask1")
                nc.vector.tensor_tensor(
                    mask1[:, :], mt[:, 2, 0:256], mt[:, 0, 0:256], mybir.AluOpType.is_gt
                )
                mk = mask1.unsqueeze(1).broadcast_to((128, 2, 256))
                nc.vector.copy_predicated(xtr[:, 0, :, 0:256], mk, xtr[:, 2, :, 0:256])

                # ---- store winners (slabs j=0 r, j=1 i; [0:256) each) ----
                nc.sync.dma_start(out=y_dram[k], in_=xtr[:, 0, :, 0:256])

    nc.compile()
    return nc


def _get_nc():
    global _COMPILED
    if _COMPILED is None:
        _COMPILED = _build()
    return _COMPILED


def _prep_core(xb: np.ndarray) -> np.ndarray:
    """xb: [2, 32, 16,16,16,16] -> [4, 128, 8192] slab-packed, parity-split."""
    # [ri, chunk, c8, o1, a, o2, b, o3, s, o4, d]
    t = xb.reshape(RI, C // NCH, NCH, O, 2, O, 2, O, 2, O, 2)
    # -> [chunk, a, b, ri, c8, o1, o2, d, s, o3, o4]
    t = t.transpose(1, 4, 6, 0, 2, 3, 5, 10, 8, 7, 9)
    # merge (o1,o2) -> split (hi, q)
    t = t.reshape(C // NCH, 2, 2, RI, NCH, 16, 4, 2, 2, O, O)
    # -> [chunk, c8, hi, a, b, ri, d, s, q, o3, o4]
    t = t.transpose(0, 4, 5, 1, 2, 3, 7, 8, 6, 9, 10)
    return np.ascontiguousarray(t).reshape(C // NCH, 128, XF)


def _post_core(y: np.ndarray) -> np.ndarray:
    """y: [4, 128, 512] -> [2, 32, 8, 8, 8, 8]."""
    # [chunk, c8, hi, ri, q, o3o4]
    yk = y.reshape(C // NCH, NCH, 16, RI, 4, O * O)
    out = yk.transpose(3, 0, 1, 2, 4, 5).reshape(RI, C, 16 * 4, O * O)
    return out.reshape(RI, C, O, O, O, O)


def _run(inputs_x: np.ndarray, trace: bool = False):
    nc = _get_nc()
    in_maps = [{"x": _prep_core(inputs_x[b])} for b in range(N_CORES)]
    last_err = None
    for _attempt in range(3):
        try:
            res = run_bass_kernel_spmd(nc, in_maps, list(range(N_CORES)), trace=trace)
            break
        except Exception as e:  # wedged-device retries
            last_err = e
            if "UNRECOVERABLE" not in str(e) and "UNAVAILABLE" not in str(e):
                raise
    else:
        raise last_err
    outs = np.empty((N_CORES, RI, C, O, O, O, O), dtype=np.float32)
    for b in range(N_CORES):
        outs[b] = _post_core(res.results[b]["y"])
    return outs, res


def kernel(input: np.ndarray) -> np.ndarray:
    input = np.asarray(input, dtype=np.float32)
    outs, _ = _run(input)
    return outs
